# revision 42
# baseline (speedup 1.0000x reference)
"""EquilibriumPropagationNet free-phase settle kernel for 8 trn2 NeuronCores.

Data-parallel over the batch: each core settles B/8 = 2048 samples.

v4 design notes:
  - The settle map u' = 0.5u + 0.5*sig'(u)*t is locally unstable (period-2 /
    chaotic) for units with large |t1|, so the trajectory must be reproduced
    at ~fp32 fidelity: fp16/bf16 anywhere in the loop (state, activations,
    matmul operands, or xW1) flips trajectories for ~100 of 163840 outputs
    and blows the absmax-rel gate.  Everything numeric is fp32 here.
  - Layout transforms (transpose, pad, pack, prescale) happen host-side.
  - u2/r2 live PACKED [128, 512]: sample block m on partitions 32m..32m+10.
    nT accumulates into a packed psum tile (1 bank) via matmul tile_position.
  - Per h-chunk k (scales: A2 = t1_const/8, w2t = W2^T/8, w2n = W2/8):
      r1  = Sigmoid(u1)                      ACT
      e4  = Square(2*r1-1) = 1-4*d1          ACT
      ntp += w2n_k^T @ r1                    PE (packed psum)
      mt  = w2t_k^T @ r2p                    PE
      t1  = mt + A2            (= t1/8)      DVE tensor_tensor (psum)
      m'  = (e4-1)*t1          (= -d1*t1/2)  scalar_tensor_tensor, DVE/GpSimd
      u1  = 0.5*u1 - m'                      scalar_tensor_tensor, DVE/GpSimd
    u2 update mirrors it straight out of psum, with b2/8 folded in via the
    scalar operand of the m2' stt (per-partition b2p4 vector).
  - mT runs as ONE fp16 matmul with K=30: moving rows [r2h; r2h; r2l] against
    stationary [W_hi; W_lo; W_hi] (split16 decomposition, lo*lo dropped at
    ~2^-22) — trajectory-exact at 1/4 the fp32 PE cost.  nT (K=128) cannot
    stack its correction terms, so it stays fp32.
"""

import numpy as np
from contextlib import ExitStack

B, D_IN, H, D_OUT = 16384, 784, 1024, 10
N_CORES = 8
BLOC = B // N_CORES   # 2048 samples per core
SC = 512              # psum-bank sample chunk
NSC = BLOC // SC      # 4
NHC = H // 128        # 8
DKP = 896             # D_IN padded to 7*128
NKC = DKP // 128      # 7

# Per-chunk update route. GpSimd cannot touch PSUM, cannot run
# scalar_tensor_tensor, and its TENSOR_SCALAR is ~30us for 2048 cols (never
# use).  'v' = fused stt update on DVE; 'g' = m' stt on DVE, u1*=0.5 on DVE
# (TS, 2x_2p), u1-=m' on GpSimd (TT, ~5us).
UPD_ROUTE = ["v", "v", "v", "v", "v", "v", "v", "v"]

_cache = {}


def _build(K: int):
    import concourse.bass as bass  # noqa: F401
    import concourse.bacc as bacc
    import concourse.mybir as mybir
    from concourse import tile, masks

    FP32 = mybir.dt.float32
    FP16 = mybir.dt.float16
    AF = mybir.ActivationFunctionType
    OP = mybir.AluOpType

    nc = bacc.Bacc(None)
    # x and W1 ship as fp16 (hi, lo) split pairs; xW1 = xh@Wh + xh@Wl + xl@Wh
    xh_d = nc.declare_dram_parameter("xh", [DKP, BLOC], FP16, isOutput=False)
    xl_d = nc.declare_dram_parameter("xl", [DKP, BLOC], FP16, isOutput=False)
    u1t_d = nc.declare_dram_parameter("u1t", [H, BLOC], FP32, isOutput=False)
    u2p_d = nc.declare_dram_parameter("u2p", [128, SC], FP32, isOutput=False)
    w1h_d = nc.declare_dram_parameter("w1h", [DKP, H], FP16, isOutput=False)
    w1l_d = nc.declare_dram_parameter("w1l", [DKP, H], FP16, isOutput=False)
    # mT stationary: K=30 stacked split [Wh; Wl; Wh] per quadrant (fp16)
    w2c_d = nc.declare_dram_parameter("w2c", [128, H], FP16, isOutput=False)
    w2n_d = nc.declare_dram_parameter("w2n", [128, NHC * D_OUT], FP32, isOutput=False)
    b1s_d = nc.declare_dram_parameter("b1s", [128, NHC], FP32, isOutput=False)
    b2p_d = nc.declare_dram_parameter("b2p", [128, 1], FP32, isOutput=False)
    y_d = nc.declare_dram_parameter("y", [BLOC, D_OUT], FP32, isOutput=True)

    with tile.TileContext(nc) as tc, ExitStack() as ctx:
        state = ctx.enter_context(tc.tile_pool(name="state", bufs=1))
        u1h = state.tile([128, NHC * BLOC], FP32, tag="u1h")
        A2 = state.tile([128, NHC * BLOC], FP32, tag="A2")
        u2p = state.tile([128, SC], FP32, tag="u2p")
        r2p = state.tile([128, SC], FP32, tag="r2p")
        # moving operand for mT: rows 32m+[0:10)=r2h, [10:20)=r2h, [20:30)=r2l
        r2hl = state.tile([128, SC], FP16, tag="r2hl")
        rlt = state.tile([128, SC], FP16, tag="rlt")   # staging for r2l
        w2c = state.tile([128, H], FP16, tag="w2c")
        w2n = state.tile([128, NHC * D_OUT], FP32, tag="w2n")
        b1s = state.tile([128, NHC], FP32, tag="b1s")
        b2p = state.tile([128, 1], FP32, tag="b2p")
        identf = state.tile([128, 128], FP32, tag="identf")
        mo = state.tile([128, 1], FP32, tag="mo")  # -1.0 bias for Square

        masks.make_identity(nc, identf[:])
        # identity replicated at each packed quadrant offset for the epilogue
        ident4 = state.tile([128, 16], FP32, tag="ident4")
        for m in range(NSC):
            nc.vector.tensor_copy(
                ident4[32 * m : 32 * m + D_OUT, :D_OUT], identf[:D_OUT, :D_OUT]
            )
        nc.gpsimd.memset(mo[:], -1.0)
        nc.vector.tensor_copy(mo[:], mo[:])

        nc.sync.dma_start(out=w2c[:], in_=w2c_d[:])
        nc.sync.dma_start(out=w2n[:], in_=w2n_d[:])
        nc.sync.dma_start(out=b1s[:], in_=b1s_d[:])
        nc.sync.dma_start(out=b2p[:], in_=b2p_d[:])

        # ---- prologue: A2 = (x@W1)^T/16 + b1/8 via fp16 split matmuls ----
        with (
            tc.tile_pool(name="w1p", bufs=1) as w1p,
            tc.tile_pool(name="xtp", bufs=2) as xtp,
            tc.tile_pool(name="pxw", bufs=8, space="PSUM") as pxw,
        ):
            w1hs = w1p.tile([128, NKC * H], FP16, tag="w1hs")
            w1ls = w1p.tile([128, NKC * H], FP16, tag="w1ls")
            nc.sync.dma_start(
                out=w1hs[:].rearrange("p (c h) -> p c h", h=H),
                in_=w1h_d[:].rearrange("(c p) h -> p c h", p=128),
            )
            nc.sync.dma_start(
                out=w1ls[:].rearrange("p (c h) -> p c h", h=H),
                in_=w1l_d[:].rearrange("(c p) h -> p c h", p=128),
            )
            for jg in range(NHC // 2):  # j-pairs; x streamed once per pair
                ps = [pxw.tile([128, SC], FP32, tag="pxw", name="pxw") for _ in range(8)]
                for c in range(NKC):
                    xhs = xtp.tile([128, BLOC], FP16, tag="xhs", name="xhs")
                    xls = xtp.tile([128, BLOC], FP16, tag="xls", name="xls")
                    nc.sync.dma_start(out=xhs[:], in_=xh_d[128 * c : 128 * (c + 1), :])
                    nc.sync.dma_start(out=xls[:], in_=xl_d[128 * c : 128 * (c + 1), :])
                    for jj in range(2):
                        j = 2 * jg + jj
                        wsl = slice(H * c + 128 * j, H * c + 128 * (j + 1))
                        for m in range(NSC):
                            msl = slice(SC * m, SC * (m + 1))
                            nc.tensor.matmul(
                                ps[4 * jj + m][:], w1hs[:, wsl], xhs[:, msl],
                                start=(c == 0), stop=False,
                            )
                            nc.tensor.matmul(
                                ps[4 * jj + m][:], w1ls[:, wsl], xhs[:, msl],
                                start=False, stop=False,
                            )
                            nc.tensor.matmul(
                                ps[4 * jj + m][:], w1hs[:, wsl], xls[:, msl],
                                start=False, stop=(c == NKC - 1),
                            )
                for jj in range(2):
                    j = 2 * jg + jj
                    for m in range(NSC):
                        nc.vector.tensor_scalar(
                            out=A2[:, BLOC * j + SC * m : BLOC * j + SC * (m + 1)],
                            in0=ps[4 * jj + m][:],
                            scalar1=0.0625,
                            scalar2=b1s[:, j : j + 1],
                            op0=OP.mult,
                            op1=OP.add,
                        )

        # state loads after the prologue frees its SBUF
        nc.sync.dma_start(out=u2p[:], in_=u2p_d[:])
        nc.sync.dma_start(
            out=u1h[:].rearrange("p (k s) -> p k s", s=BLOC),
            in_=u1t_d[:].rearrange("(k p) s -> p k s", p=128),
        )
        def emit_r2hl():
            # rows 32m+[0:10) of r2hl = fp16(r2p); rlt = residual r2p - r2hl
            nc.scalar.activation(r2hl[:], r2p[:], AF.Copy)
            nc.vector.tensor_tensor(out=rlt[:], in0=r2p[:], in1=r2hl[:], op=OP.subtract)
            # partition-shift via SBUF->SBUF DMA: dup r2h to [10:20), r2l to [20:30)
            for m in range(NSC):
                nc.sync.dma_start(
                    out=r2hl[32 * m + 10 : 32 * m + 20, :],
                    in_=r2hl[32 * m : 32 * m + 10, :],
                )
                nc.sync.dma_start(
                    out=r2hl[32 * m + 20 : 32 * m + 30, :],
                    in_=rlt[32 * m : 32 * m + 10, :],
                )

        nc.scalar.activation(r2p[:], u2p[:], AF.Sigmoid)
        emit_r2hl()

        # ---- settle steps ----
        r1p = ctx.enter_context(tc.tile_pool(name="r1p", bufs=2))
        wkp = ctx.enter_context(tc.tile_pool(name="wkp", bufs=3))
        t1p = ctx.enter_context(tc.tile_pool(name="t1p", bufs=2))
        wu2p = ctx.enter_context(tc.tile_pool(name="wu2p", bufs=2))
        psn = ctx.enter_context(tc.tile_pool(name="psn", bufs=2, space="PSUM"))
        psm = ctx.enter_context(tc.tile_pool(name="psm", bufs=2, space="PSUM"))
        psy = ctx.enter_context(tc.tile_pool(name="psy", bufs=2, space="PSUM"))
        ENG = {"v": nc.vector, "g": nc.gpsimd}

        for t in range(K):
            ntp = psn.tile([128, SC], FP32, tag="ntp", name="ntp")
            for k in range(NHC):
                u1k = u1h[:, BLOC * k : BLOC * (k + 1)]
                r1 = r1p.tile([128, BLOC], FP32, tag="r1", name="r1")
                nc.scalar.activation(r1[:], u1k, AF.Sigmoid)
                for m in range(NSC):
                    nc.tensor.matmul(
                        ntp[32 * m : 32 * m + D_OUT, :],
                        w2n[:, D_OUT * k : D_OUT * (k + 1)],
                        r1[:, SC * m : SC * (m + 1)],
                        start=(k == 0),
                        stop=(k == NHC - 1),
                        tile_position=(0, 32 * m),
                    )
                wk = wkp.tile([128, BLOC], FP32, tag="wk", name="wk")
                nc.scalar.activation(wk[:], r1[:], AF.Square, bias=mo[:], scale=2.0)
                t1 = t1p.tile([128, BLOC], FP32, tag="t1", name="t1")
                for hh in range(2):
                    mt = psm.tile([128, 2 * SC], FP32, tag="mt", name="mt")
                    for mm in range(2):
                        m_ = 2 * hh + mm
                        nc.tensor.matmul(
                            mt[:, SC * mm : SC * (mm + 1)],
                            w2c[32 * m_ : 32 * m_ + 30, 128 * k : 128 * (k + 1)],
                            r2hl[32 * m_ : 32 * m_ + 30, :],
                            start=True,
                            stop=True,
                            tile_position=(32 * m_, 0),
                        )
                    nc.vector.tensor_tensor(
                        out=t1[:, 2 * SC * hh : 2 * SC * (hh + 1)],
                        in0=mt[:],
                        in1=A2[:, BLOC * k + 2 * SC * hh : BLOC * k + 2 * SC * (hh + 1)],
                        op=OP.add,
                    )
                # m' = (e4 - 1) * t1  (= -d1*t1/2), in place on wk
                nc.vector.scalar_tensor_tensor(
                    out=wk[:], in0=wk[:], scalar=1.0, in1=t1[:],
                    op0=OP.subtract, op1=OP.mult,
                )
                # u1 = 0.5*u1 - m'
                if UPD_ROUTE[k] == "v":
                    nc.vector.scalar_tensor_tensor(
                        out=u1k, in0=u1k, scalar=0.5, in1=wk[:],
                        op0=OP.mult, op1=OP.subtract,
                    )
                else:
                    nc.vector.tensor_scalar(
                        out=u1k, in0=u1k, scalar1=0.5, scalar2=None, op0=OP.mult
                    )
                    nc.gpsimd.tensor_tensor(out=u1k, in0=u1k, in1=wk[:], op=OP.subtract)
            # ---- packed u2 update: m2' = (ntp + b2p)*(e42-1) per quadrant ----
            wu = wu2p.tile([128, SC], FP32, tag="wu", name="wu")
            nc.scalar.activation(wu[:], r2p[:], AF.Square, bias=mo[:], scale=2.0)
            nc.vector.tensor_scalar(
                out=wu[:], in0=wu[:], scalar1=1.0, scalar2=None, op0=OP.subtract
            )
            for m in range(NSC):
                sl = slice(32 * m, 32 * m + D_OUT)
                nc.vector.scalar_tensor_tensor(
                    out=wu[sl, :], in0=ntp[sl, :], scalar=b2p[sl, :], in1=wu[sl, :],
                    op0=OP.add, op1=OP.mult,
                )
            nc.vector.scalar_tensor_tensor(
                out=u2p[:], in0=u2p[:], scalar=0.5, in1=wu[:],
                op0=OP.mult, op1=OP.subtract,
            )
            nc.scalar.activation(r2p[:], u2p[:], AF.Sigmoid)
            emit_r2hl()

        # ---- epilogue: y = sigmoid(u2) == r2p, unpack to [BLOC, 10] ----
        with tc.tile_pool(name="yout", bufs=3) as yout:
            for m in range(NSC):
                for i in range(SC // 128):
                    pt = psy.tile([128, 128], FP32, tag="pty", name="pty")
                    nc.tensor.transpose(
                        pt[:, :D_OUT],
                        r2p[32 * m : 32 * m + D_OUT, 128 * i : 128 * (i + 1)],
                        ident4[32 * m : 32 * m + D_OUT, :D_OUT],
                        tile_position=(32 * m, 0),
                    )
                    yt = yout.tile([128, D_OUT], FP32, tag="yt", name="yt")
                    nc.vector.tensor_copy(yt[:], pt[:, :D_OUT])
                    nc.sync.dma_start(
                        out=y_d[SC * m + 128 * i : SC * m + 128 * (i + 1), :], in_=yt[:]
                    )

    return nc


def _prep_core(x_c, u1_c, u2_c):
    xt = np.zeros((DKP, BLOC), dtype=np.float32)
    xt[:D_IN] = x_c.T
    xh = xt.astype(np.float16)
    xl = (xt - xh.astype(np.float32)).astype(np.float16)
    u1t = np.ascontiguousarray(u1_c.T)
    u2p = np.zeros((128, SC), dtype=np.float32)
    for m in range(NSC):
        u2p[32 * m : 32 * m + D_OUT] = u2_c[SC * m : SC * (m + 1)].T
    return xh, xl, u1t, u2p


def _split16(v):
    hi = v.astype(np.float16)
    lo = (v - hi.astype(np.float32)).astype(np.float16)
    return hi, lo


def run(inputs: dict, trace: bool = False):
    from concourse.bass_utils import run_bass_kernel_spmd

    K = int(inputs["steps"])
    if K not in _cache:
        nc = _build(K)
        if not nc.is_finalized():
            nc.finalize()
        _cache[K] = nc
    nc = _cache[K]

    x = np.asarray(inputs["x"], dtype=np.float32)
    u1 = np.asarray(inputs["u1"], dtype=np.float32)
    u2 = np.asarray(inputs["u2"], dtype=np.float32)
    W1 = np.asarray(inputs["W1"], dtype=np.float32)
    W2 = np.asarray(inputs["W2"], dtype=np.float32)
    b1 = np.asarray(inputs["b1"], dtype=np.float32)
    b2 = np.asarray(inputs["b2"], dtype=np.float32)

    w1p = np.zeros((DKP, H), dtype=np.float32)
    w1p[:D_IN] = W1
    w1h = w1p.astype(np.float16)
    w1l = (w1p - w1h.astype(np.float32)).astype(np.float16)

    w2t_s = (W2.T / 8.0).astype(np.float32)          # [10, 1024]
    w2n_s = (W2 / 8.0).astype(np.float32)            # [1024, 10]
    # mT stationary: quadrant rows [0:10)=W_hi, [10:20)=W_lo, [20:30)=W_hi
    th, tl = _split16(w2t_s)
    w2c = np.zeros((128, H), dtype=np.float16)
    for m in range(NSC):
        w2c[32 * m : 32 * m + D_OUT] = th
        w2c[32 * m + D_OUT : 32 * m + 2 * D_OUT] = tl
        w2c[32 * m + 2 * D_OUT : 32 * m + 3 * D_OUT] = th
    w2n = np.ascontiguousarray(
        w2n_s.reshape(NHC, 128, D_OUT).transpose(1, 0, 2).reshape(128, NHC * D_OUT)
    )

    b1s = np.ascontiguousarray((b1 / 8.0).reshape(NHC, 128).T).astype(np.float32)
    b2p = np.zeros((128, 1), dtype=np.float32)
    for m in range(NSC):
        b2p[32 * m : 32 * m + D_OUT, 0] = b2 / 8.0

    in_maps = []
    for c in range(N_CORES):
        s = slice(c * BLOC, (c + 1) * BLOC)
        xh, xl, u1t, u2pp = _prep_core(x[s], u1[s], u2[s])
        in_maps.append(
            {
                "xh": xh,
                "xl": xl,
                "u1t": u1t,
                "u2p": u2pp,
                "w1h": w1h,
                "w1l": w1l,
                "w2c": w2c,
                "w2n": w2n,
                "b1s": b1s,
                "b2p": b2p,
            }
        )
    res = run_bass_kernel_spmd(nc, in_maps, list(range(N_CORES)), trace=trace)
    y = np.concatenate([res.results[c]["y"] for c in range(N_CORES)], axis=0)
    return y.astype(np.float32), res


def kernel(**inputs) -> np.ndarray:
    y, _ = run(inputs, trace=False)
    return y
